# revision 18
# baseline (speedup 1.0000x reference)
"""Trainium2 Bass kernel for nn_MultiHeadDilatedState.

Sharding: data-parallel over batch (B=8 -> 8 cores, one sequence per core).
Weights replicated. Per-core dataflow is channel-major [768, 4096], fp16
activations with fp32 PSUM accumulation:

  x16 [S,H] --fp16 PE transpose--> xt16 [H,S] --fp16 matmul--> GLU + router
  conv stages: per-head 2-tap-packed diagonal matmuls. For each (stage, head)
  a "stack" tile [128, S] holds [h ; shift_{2d}(h)] built by SBUF->SBUF DMA
  from the fp16 hbuf state; one matmul with a [128, 64] dual-diagonal
  stationary computes taps {0,2}, a second computes taps {1,3} (moving AP
  offset -d). Residual+bias folded into the DVE scalar_tensor_tensor
  evacuation, in-place over fp16 hbuf. Head-weight gating via a 12->768
  replication matmul from SBUF-resident router outputs, mix-gate matmul,
  final matmul with the activation as the stationary operand so the output
  comes out token-major.
"""

import os
import numpy as np

import concourse.bass as bass
import concourse.bacc as bacc
import concourse.mybir as mybir
import concourse.tile as tile
from concourse.bass_utils import run_bass_kernel_spmd
from concourse.masks import make_identity

B, S, HID = 8, 4096, 768
NH, HD, KT = 12, 64, 4  # heads, head_dim, kernel taps
NC = 6                  # 768 / 128 channel chunks
ST = 512                # token tile
NST = S // ST           # 8
F32 = mybir.dt.float32
F16 = mybir.dt.float16
SIG = mybir.ActivationFunctionType.Sigmoid
IDENT = mybir.ActivationFunctionType.Identity
ADD = mybir.AluOpType.add

DILATIONS = [(1, 2, 4), (1, 1, 1), (4, 8, 16), (8, 16, 32), (32, 64, 128),
             (64, 128, 256), (256, 512, 1024), (1, 100, 200), (1, 500, 1000),
             (1, 1024, 2048), (3, 9, 27), (5, 25, 125)]


def build_bass():
    nc = bacc.Bacc()

    x_d = nc.dram_tensor("xb16", [S, HID], F16, kind="ExternalInput")
    gwT_d = nc.dram_tensor("gwT16", [128, NC, 2 * HID], F16, kind="ExternalInput")
    mgw_d = nc.dram_tensor("mgw16", [128, NC, HID], F16, kind="ExternalInput")
    rwr_d = nc.dram_tensor("rwr16", [128, NC, 64], F16, kind="ExternalInput")
    rb_d = nc.dram_tensor("rb", [NH, 1], F32, kind="ExternalInput")
    cvstat_d = nc.dram_tensor("cvstat", [128, 72, HD], F16, kind="ExternalInput")
    convbias_d = nc.dram_tensor("convbias", [128, NC, 16], F32, kind="ExternalInput")
    erep_d = nc.dram_tensor("erep16", [NH, NC, 128], F16, kind="ExternalInput")
    mgb_d = nc.dram_tensor("mgb", [128, NC], F32, kind="ExternalInput")
    mixbias_d = nc.dram_tensor("mixbias", [128, HID], F32, kind="ExternalInput")
    mixt16_d = nc.dram_tensor("mixt16", [128, NC, HID], F16, kind="ExternalInput")
    out_d = nc.dram_tensor("out", [S, HID], F32, kind="ExternalOutput")
    dbg_d = nc.dram_tensor("dbg", [NC, 128, S], F16, kind="ExternalOutput") if os.environ.get("KDBG") else None

    with tile.TileContext(nc) as tc:
        _body(tc, x_d, gwT_d, mgw_d, rwr_d, rb_d, cvstat_d,
              convbias_d, erep_d, mgb_d, mixbias_d, mixt16_d, out_d, dbg_d)
    nc.finalize()
    return nc


def _body(tc, x_d, gwT_d, mgw_d, rwr_d, rb_d, cvstat_d,
          convbias_d, erep_d, mgb_d, mixbias_d, mixt16_d, out_d, dbg_d=None):
    nc = tc.nc

    with (
        tc.tile_pool(name="persist", bufs=1) as persist,
        tc.tile_pool(name="xload", bufs=3) as p_xload,
        tc.tile_pool(name="xt", bufs=3) as p_xt,
        tc.tile_pool(name="stk", bufs=6) as p_stk,
        tc.tile_pool(name="sig", bufs=4) as p_sig,
        tc.tile_pool(name="outsb", bufs=2) as p_out,
    ):
        # ---- persistent weights ----
        # (128B-aligned tiles first: fp16 LDWEIGHTS at SBUF addresses not
        # 0 mod 128 load corrupted weights into array columns 64+.)
        gwT = persist.tile([128, NC, 2 * HID], F16, tag="bigw")
        cvstat = persist.tile([128, 72, HD], F16, tag="cvstat")
        mixt16 = persist.tile([128, NC, HID], F16, tag="mixt16")
        rwr = persist.tile([128, NC, 64], F16, tag="rwr")
        erep = persist.tile([NH, NC, 128], F16, tag="erep")
        mixbias = persist.tile([128, HID], F32, tag="mixbias")
        ident = persist.tile([128, 128], F16, tag="ident")
        make_identity(nc, ident[:, :])
        hbuf = [persist.tile([128, S], F16, tag=f"h{c}", name=f"h{c}")
                for c in range(NC)]
        hw16 = persist.tile([NH, S], F16, tag="hw16")
        rb_p = persist.tile([NH, 32], F32, tag="rb")
        rb = rb_p[:, 0:1]
        convbias = persist.tile([128, NC, 16], F32, tag="convbias")
        mgb_p = persist.tile([128, 32], F32, tag="mgb")
        mgb = mgb_p[:, 0:NC]
        # critical-path weights first (router + GLU)
        nc.sync.dma_start(rwr, rwr_d[:, :, :])
        nc.sync.dma_start(rb, rb_d[:, :])
        nc.sync.dma_start(gwT, gwT_d[:, :, :])

        use_pe_transpose = os.environ.get("KTRANS") == "pe"

        # ---- phase A: transpose + router + GLU ----
        # xbar DMA transposes straight from DRAM into a full-S xT buffer,
        # all emitted up front (PE consumes tile st only ~15us/st later).
        xtF = persist.tile([128, NC, S], F16, tag="xtF")
        with tc.tile_pool(name="psA", bufs=1, space="PSUM") as psA:
            if not use_pe_transpose:
                for st in range(NST):
                    s0 = st * ST
                    for kc in range(NC):
                        nc.sync.dma_start(xtF[:, kc, s0:s0 + ST],
                                          x_d[s0:s0 + ST, kc * 128:(kc + 1) * 128],
                                          transpose=True)
            # non-critical weights load behind the transposes
            nc.sync.dma_start(cvstat, cvstat_d[:, :, :])
            nc.sync.dma_start(convbias, convbias_d[:, :, :])
            nc.sync.dma_start(erep, erep_d[:, :, :])
            nc.sync.dma_start(mixt16, mixt16_d[:, :, :])
            nc.sync.dma_start(mixbias, mixbias_d[:, :])
            nc.sync.dma_start(mgb, mgb_d[:, :])
            for st in range(NST):
                s0 = st * ST
                if use_pe_transpose:
                    xt = p_xt.tile([128, NC, ST], F16, tag="xt")
                    for sub in range(4):
                        xs = p_xload.tile([128, HID], F16, tag="xs")
                        nc.sync.dma_start(xs, x_d[s0 + sub * 128: s0 + (sub + 1) * 128, :])
                        for kc in range(NC):
                            ptp = psA.tile([128, 128], F16, tag="tp", bufs=2)
                            nc.tensor.transpose(ptp[:, :], xs[:, kc * 128:(kc + 1) * 128],
                                                ident[:, :])
                            nc.scalar.copy(xt[:, kc, sub * 128:(sub + 1) * 128],
                                           ptp[:, :])
                else:
                    xt = xtF[:, :, s0:s0 + ST]
                # router -> sigmoid -> keep head weights in SBUF
                pr = psA.tile([NH, ST], F32, tag="rtr", bufs=2)
                for kc in range(NC):
                    nc.tensor.matmul(pr[:, :], rwr[:, kc, 0:NH], xt[:, kc, :],
                                     start=(kc == 0), stop=(kc == NC - 1))
                nc.scalar.activation(hw16[:, s0:s0 + ST], pr[:, :], SIG,
                                     bias=rb[:, :], scale=1.0)
                # GLU
                for oc in range(NC):
                    pg = psA.tile([128, ST], F32, tag="glu", bufs=4)
                    for kc in range(NC):
                        nc.tensor.matmul(
                            pg[:, :],
                            gwT[:, kc, HID + oc * 128: HID + (oc + 1) * 128],
                            xt[:, kc, :],
                            start=(kc == 0), stop=(kc == NC - 1))
                    sg = p_sig.tile([128, ST], F16, tag="sig")
                    nc.scalar.activation(sg[:, :], pg[:, :], SIG)
                    pv = psA.tile([128, ST], F32, tag="glu", bufs=4)
                    for kc in range(NC):
                        nc.tensor.matmul(
                            pv[:, :],
                            gwT[:, kc, oc * 128:(oc + 1) * 128],
                            xt[:, kc, :],
                            start=(kc == 0), stop=(kc == NC - 1))
                    nc.vector.tensor_mul(hbuf[oc][:, s0:s0 + ST], pv[:, :], sg[:, :])

        if dbg_d is not None and os.environ.get("KDBG") == "A":
            for c in range(NC):
                nc.sync.dma_start(dbg_d[c, :, :], hbuf[c][:, :])

        # ---- phase B: 3 conv stages, in-place over fp16 hbuf ----
        # Per (stage, head): stack tile [h ; shift_{2d}(h)] via SBUF->SBUF DMA,
        # then 2 matmuls per token tile: [128,64] dual-diag stationaries pack
        # taps {0,2} (moving offset 0) and taps {1,3} (moving offset -d).
        nstages = int(os.environ.get('KSTAGES', '3'))
        with tc.tile_pool(name="psB", bufs=1, space="PSUM") as psB:
            for j in range(nstages):
                for c in range(NC):
                    stks = []
                    for half in (0, 1):
                        head = 2 * c + half
                        d = DILATIONS[head][j]
                        p0 = 64 * half
                        stk = p_stk.tile([128, S], F16, tag="stk",
                                         name=f"stk{j}_{head}")
                        z = min(2 * d, S)
                        nc.gpsimd.memset(stk[64:128, 0:z], 0)
                        # lower half: plain copy of this head's state (3 pieces)
                        for q in range(3):
                            a0, a1 = q * S // 3, (q + 1) * S // 3
                            nc.gpsimd.dma_start(stk[0:64, a0:a1],
                                                hbuf[c][p0:p0 + 64, a0:a1])
                        # upper half: shifted by 2d (up to 3 pieces)
                        n = S - z
                        for q in range(3):
                            a0, a1 = q * n // 3, (q + 1) * n // 3
                            if a1 > a0:
                                nc.scalar.dma_start(stk[64:128, z + a0:z + a1],
                                                    hbuf[c][p0:p0 + 64, a0:a1])
                        stks.append(stk)
                    for st in range(NST):
                        s0 = st * ST
                        pc = psB.tile([128, ST], F32, tag="conv",
                                      name=f"cv{j}_{c}_{st}", bufs=4)
                        for half in (0, 1):
                            head = 2 * c + half
                            d = DILATIONS[head][j]
                            p0 = 64 * half
                            u0 = (j * NH + head) * 2
                            stk = stks[half]
                            mm2 = d < s0 + ST
                            nc.tensor.matmul(
                                pc[p0:p0 + 64, 0:ST],
                                cvstat[:, u0, :],
                                stk[:, s0:s0 + ST],
                                start=True, stop=not mm2,
                                tile_position=(0, p0))
                            if mm2:
                                a = max(0, d - s0)
                                nc.tensor.matmul(
                                    pc[p0:p0 + 64, a:ST],
                                    cvstat[:, u0 + 1, :],
                                    stk[:, s0 - d + a: s0 - d + ST],
                                    start=False, stop=True,
                                    tile_position=(0, p0))
                        # residual is folded into tap0 weights; evac = psum+bias,
                        # alternating DVE/ACT to split the load
                        if st % 2 == 0:
                            nc.vector.tensor_scalar_add(
                                hbuf[c][:, s0:s0 + ST], pc[:, :],
                                convbias[:, c, j:j + 1])
                        else:
                            nc.scalar.activation(
                                hbuf[c][:, s0:s0 + ST], pc[:, :], IDENT,
                                bias=convbias[:, c, j:j + 1], scale=1.0)
                    if j == nstages - 1:
                        # ---- phase B2 for this chunk: head-weight gating ----
                        for st in range(NST):
                            s0 = st * ST
                            ph = psB.tile([128, ST], F32, tag="hwr", bufs=3)
                            nc.tensor.matmul(ph[:, :], erep[:, c, :],
                                             hw16[:, s0:s0 + ST],
                                             start=True, stop=True)
                            nc.vector.tensor_mul(hbuf[c][:, s0:s0 + ST],
                                                 hbuf[c][:, s0:s0 + ST], ph[:, :])

            if dbg_d is not None and os.environ.get("KDBG") == "B":
                for c in range(NC):
                    nc.sync.dma_start(dbg_d[c, :, :], hbuf[c][:, :])

        # load mix-gate weights into the slot gwT used (same tag -> same space)
        mgw = persist.tile([128, NC, HID], F16, tag="bigw", name="mgw")
        nc.sync.dma_start(mgw, mgw_d[:, :, :])

        # out2 tiles reuse the xT slot (xT fully consumed by GLU/router)
        o16F = persist.tile([128, NC, S], F16, tag="xtF", name="o16F")

        with tc.tile_pool(name="psC", bufs=1, space="PSUM") as psC:
            # ---- phase C: mix gate -> fp16 out2 tiles; D: final matmul ----
            for st in range(NST):
                s0 = st * ST
                o16 = o16F[:, :, s0:s0 + ST]
                for oc in range(NC):
                    pm = psC.tile([128, ST], F32, tag="mg", bufs=3)
                    for kc in range(NC):
                        nc.tensor.matmul(
                            pm[:, :],
                            mgw[:, kc, oc * 128:(oc + 1) * 128],
                            hbuf[kc][:, s0:s0 + ST],
                            start=(kc == 0), stop=(kc == NC - 1))
                    sg = p_sig.tile([128, ST], F16, tag="sig")
                    nc.scalar.activation(sg[:, :], pm[:, :], SIG,
                                         bias=mgb[:, oc:oc + 1], scale=1.0)
                    nc.vector.tensor_mul(o16[:, oc, :],
                                         hbuf[oc][:, s0:s0 + ST], sg[:, :])

                # ---- phase D on this 512-token block (fp16 inputs) ----
                for tl in range(4):
                    c0 = s0 + tl * 128
                    pmx = psC.tile([128, HID], F32, tag="mx", bufs=2)
                    for kc in range(NC):
                        nc.tensor.matmul(pmx[:, 0:512],
                                         o16[:, kc, tl * 128:(tl + 1) * 128],
                                         mixt16[:, kc, 0:512],
                                         start=(kc == 0), stop=(kc == NC - 1))
                    for kc in range(NC):
                        nc.tensor.matmul(pmx[:, 512:HID],
                                         o16[:, kc, tl * 128:(tl + 1) * 128],
                                         mixt16[:, kc, 512:HID],
                                         start=(kc == 0), stop=(kc == NC - 1))
                    osb = p_out.tile([128, HID], F32, tag="osb")
                    nc.vector.tensor_add(osb[:, :], pmx[:, :], mixbias[:, :])
                    nc.sync.dma_start(out_d[c0:c0 + 128, :], osb[:, :])


def _prep_weights(gate_w, conv_w, conv_b, router_w, router_b,
                  mix_gate_w, mix_gate_b, mixing_w, mixing_b):
    f = np.float32
    h = np.float16
    gwT16 = np.ascontiguousarray(
        gate_w.T.reshape(NC, 128, 2 * HID).transpose(1, 0, 2), dtype=h)
    mgw16 = np.ascontiguousarray(
        mix_gate_w.T.reshape(NC, 128, HID).transpose(1, 0, 2), dtype=h)
    rwr = np.zeros((128, NC, 64), dtype=h)
    rwr[:, :, 0:NH] = router_w.T.reshape(NC, 128, NH).transpose(1, 0, 2)
    rb = np.ascontiguousarray(router_b.reshape(NH, 1), dtype=f)

    # conv stationaries: unit u = (j*12+head)*2 + m; m=0 packs taps {0,2}
    # (weights 1+w[:,3] — residual folded into tap0 — and w[:,1]),
    # m=1 packs taps {1,3} (w[:,2], w[:,0]).
    cv = np.zeros((128, 72, HD), dtype=h)
    ar = np.arange(HD)
    for j in range(3):
        for head in range(NH):
            u0 = (j * NH + head) * 2
            w = conv_w[head, j]  # [HD, KT]
            cv[ar, u0, ar] = 1.0 + w[:, 3]
            cv[HD + ar, u0, ar] = w[:, 1]
            cv[ar, u0 + 1, ar] = w[:, 2]
            cv[HD + ar, u0 + 1, ar] = w[:, 0]
    cvstat = np.ascontiguousarray(cv)

    cb = np.zeros((128, NC, 16), dtype=f)
    for c in range(NC):
        for half in (0, 1):
            cb[half * HD:(half + 1) * HD, c, 0:3] = conv_b[2 * c + half].T
    convbias = np.ascontiguousarray(cb)

    er = np.zeros((NH, NC, 128), dtype=h)
    for c in range(NC):
        for m in range(128):
            er[2 * c + (m >= HD), c, m] = 1.0

    mgb = np.ascontiguousarray(mix_gate_b.reshape(NC, 128).T, dtype=f)
    mixt16 = np.ascontiguousarray(
        mixing_w.T.astype(h).reshape(NC, 128, HID).transpose(1, 0, 2))
    mixbias = np.ascontiguousarray(np.tile(mixing_b[None, :], (128, 1)), dtype=f)

    return {"gwT16": gwT16, "mgw16": mgw16, "rwr16": rwr, "rb": rb,
            "cvstat": cvstat, "convbias": convbias,
            "erep16": er, "mgb": mgb, "mixbias": mixbias, "mixt16": mixt16}


_CACHE = {}


def _run(inputs, trace=False, tmpdir=None):
    if "nc" not in _CACHE:
        _CACHE["nc"] = build_bass()
    nc = _CACHE["nc"]

    w = _prep_weights(
        np.asarray(inputs["gate_w"]), np.asarray(inputs["conv_w"]),
        np.asarray(inputs["conv_b"]), np.asarray(inputs["router_w"]),
        np.asarray(inputs["router_b"]), np.asarray(inputs["mix_gate_w"]),
        np.asarray(inputs["mix_gate_b"]), np.asarray(inputs["mixing_w"]),
        np.asarray(inputs["mixing_b"]))
    x = np.asarray(inputs["x"])

    in_maps = [dict(w, xb16=np.ascontiguousarray(x[b], dtype=np.float16))
               for b in range(B)]
    res = run_bass_kernel_spmd(nc, in_maps, core_ids=list(range(B)),
                               trace=trace, tmpdir=tmpdir)
    out = np.stack([res.results[b]["out"] for b in range(B)], axis=0)
    return out, res


def kernel(**inputs):
    out, _ = _run(inputs, trace=False)
    return out


if __name__ == "__main__":
    nc = build_bass()
    print("built ok; instructions:", len(nc.inst_map))
